# revision 4
# baseline (speedup 1.0000x reference)
"""EMA (first-order linear recurrence along T) for x[16, 512, 4096] f32.

y[..., 0] = x[..., 0];  y[..., t] = s_c * x[..., t] + (1 - s_c) * y[..., t-1]

Sharding: data-parallel over batch B across 8 cores (2 batches/core, each a
contiguous 16 MiB slab). Per core the (b, c) pairs form 1024 independent rows
of length T=4096; the recurrence maps 1:1 onto the DVE's TensorTensorScanArith
instruction (state = data0*state + data1 along the free dim, one recurrence per
partition). The s_c*x premultiply runs on the scalar engine (ACTIVATE Copy
with per-partition scale) so the DVE only does scans; the kernel is DMA-bound.
"""

import numpy as np

import concourse.bacc as bacc
import concourse.mybir as mybir
import concourse.tile as tile
from concourse.bass_utils import run_bass_kernel_spmd

B, C, T = 16, 512, 4096
N_CORES = 8
B_PER = B // N_CORES          # 2 batches per core
ROWS = B_PER * C              # 1024 (b, c) rows per core
P = 128                       # SBUF partitions
N_BLOCKS = ROWS // P          # 8 row blocks per core
C_BLOCKS = C // P             # 4 channel blocks (weights layout)

DT = mybir.dt.float32
OP = mybir.AluOpType


def build(b_per=B_PER, c=C, t=T):
    rows = b_per * c
    n_blocks = rows // P
    c_blocks = c // P

    nc = bacc.Bacc("TRN2", target_bir_lowering=False, debug=False)

    x_in = nc.dram_tensor("x", [b_per, c, t], DT, kind="ExternalInput")
    w_in = nc.dram_tensor("weights", [c], DT, kind="ExternalInput")
    y_out = nc.dram_tensor("out", [b_per, c, t], DT, kind="ExternalOutput")

    xr = x_in.ap().rearrange("b c t -> (b c) t")   # [rows, t]
    yr = y_out.ap().rearrange("b c t -> (b c) t")
    # w4[p, j] = weights[j*128 + p] — column j holds channel block j
    wr = w_in.ap().rearrange("(j p) -> p j", p=P)  # [128, c_blocks]

    with tile.TileContext(nc) as tc:
        with (
            tc.tile_pool(name="const", bufs=1) as cpool,
            tc.tile_pool(name="xp", bufs=3) as xpool,
            tc.tile_pool(name="sp", bufs=3) as spool,
            tc.tile_pool(name="yp", bufs=3) as ypool,
        ):
            w4 = cpool.tile([P, c_blocks], DT)
            s4 = cpool.tile([P, c_blocks], DT)
            a4 = cpool.tile([P, c_blocks], DT)
            nc.sync.dma_start(w4[:], wr)
            # s = clamp(w, 0, 1); a = 1 - s
            nc.vector.tensor_scalar(s4[:], w4[:], 0.0, 1.0, OP.max, OP.min)
            nc.vector.tensor_scalar(a4[:], s4[:], -1.0, 1.0, OP.mult, OP.add)

            for k in range(n_blocks):
                j = k % c_blocks  # channel block of rows [k*128, (k+1)*128)
                xt = xpool.tile([P, t], DT)
                nc.sync.dma_start(xt[:], xr[k * P:(k + 1) * P, :])

                st = spool.tile([P, t], DT)
                nc.scalar.activation(
                    st[:], xt[:], mybir.ActivationFunctionType.Copy,
                    scale=s4[:, j:j + 1],
                )

                yt = ypool.tile([P, t], DT)
                # state_0 = a*x_0 + s*x_0 = x_0; state_t = a*state + s*x_t
                nc.vector.tensor_tensor_scan(
                    yt[:],
                    a4[:, j:j + 1].to_broadcast((P, t)),
                    st[:],
                    xt[:, 0:1],
                    OP.mult,
                    OP.add,
                )
                nc.sync.dma_start(yr[k * P:(k + 1) * P, :], yt[:])
    nc.compile()
    return nc


_NC_CACHE = []


def kernel(x, weights, _run_kwargs=None):
    if not _NC_CACHE:
        _NC_CACHE.append(build())
    nc = _NC_CACHE[0]
    x = np.ascontiguousarray(np.asarray(x, dtype=np.float32))
    weights = np.ascontiguousarray(np.asarray(weights, dtype=np.float32))
    in_maps = [
        {"x": x[i * B_PER:(i + 1) * B_PER], "weights": weights}
        for i in range(N_CORES)
    ]
    res = run_bass_kernel_spmd(
        nc, in_maps, core_ids=list(range(N_CORES)), **(_run_kwargs or {})
    )
    out = np.concatenate([res.results[i]["out"] for i in range(N_CORES)], axis=0)
    if _run_kwargs:
        kernel.last_results = res
    return out


# revision 6
# speedup vs baseline: 1.0041x; 1.0041x over previous
"""EMA (first-order linear recurrence along T) for x[16, 512, 4096] f32.

y[..., 0] = x[..., 0];  y[..., t] = s_c * x[..., t] + (1 - s_c) * y[..., t-1]

Sharding: data-parallel over batch B across 8 cores (2 batches/core, each a
contiguous 16 MiB slab). Per core the (b, c) pairs form 1024 independent rows
of length T=4096; the recurrence maps 1:1 onto the TensorTensorScanArith
instruction (state = data0*state + data1 along the free dim, one recurrence
per partition).

Pipeline per 128-row block, all in-place on one SBUF tile X:
  DMA in -> ACT: X[:,1:] *= s (per-partition scale, scalar engine)
         -> scan: X = a*state + X with initial=0 (col 0 still holds raw x_0,
            so state_0 = x_0 exactly) -> DMA out.
Scans alternate Vector/GpSimd so neither engine reaches the DMA roofline
(~94 us for 33.5 MB per core); a lone Vector doing all 8 scans would sit at
~93 us busy and fight the DMA for the critical path.
"""

import numpy as np

import concourse.bacc as bacc
import concourse.mybir as mybir
import concourse.tile as tile
from concourse.bass_utils import run_bass_kernel_spmd

B, C, T = 16, 512, 4096
N_CORES = 8
B_PER = B // N_CORES          # 2 batches per core
ROWS = B_PER * C              # 1024 (b, c) rows per core
P = 128                       # SBUF partitions
N_BLOCKS = ROWS // P          # 8 row blocks per core
C_BLOCKS = C // P             # 4 channel blocks (weights layout)

DT = mybir.dt.float32
OP = mybir.AluOpType

SPLIT_SCAN_ENGINES = False    # GpSimd lacks the scan opcode on trn2 (ISA check)
BUFS = 8


def build(b_per=B_PER, c=C, t=T):
    rows = b_per * c
    n_blocks = rows // P
    c_blocks = c // P

    nc = bacc.Bacc("TRN2", target_bir_lowering=False, debug=False)

    x_in = nc.dram_tensor("x", [b_per, c, t], DT, kind="ExternalInput")
    w_in = nc.dram_tensor("weights", [c], DT, kind="ExternalInput")
    y_out = nc.dram_tensor("out", [b_per, c, t], DT, kind="ExternalOutput")

    xr = x_in.ap().rearrange("b c t -> (b c) t")   # [rows, t]
    yr = y_out.ap().rearrange("b c t -> (b c) t")
    # w4[p, j] = weights[j*128 + p] — column j holds channel block j
    wr = w_in.ap().rearrange("(j p) -> p j", p=P)  # [128, c_blocks]

    with tile.TileContext(nc) as tc:
        with (
            tc.tile_pool(name="const", bufs=1) as cpool,
            tc.tile_pool(name="xp", bufs=BUFS) as xpool,
        ):
            w4 = cpool.tile([P, c_blocks], DT)
            s4 = cpool.tile([P, c_blocks], DT)
            a4 = cpool.tile([P, c_blocks], DT)
            nc.sync.dma_start(w4[:], wr)
            # s = clamp(w, 0, 1); a = 1 - s
            nc.vector.tensor_scalar(s4[:], w4[:], 0.0, 1.0, OP.max, OP.min)
            nc.vector.tensor_scalar(a4[:], s4[:], -1.0, 1.0, OP.mult, OP.add)

            for k in range(n_blocks):
                j = k % c_blocks  # channel block of rows [k*128, (k+1)*128)
                xt = xpool.tile([P, t], DT)
                nc.sync.dma_start(xt[:], xr[k * P:(k + 1) * P, :])

                # Premultiply s*x in place, skipping col 0: the scan's t=0
                # step then computes state_0 = a*0 + x_0 = x_0 exactly.
                nc.scalar.activation(
                    xt[:, 1:t], xt[:, 1:t], mybir.ActivationFunctionType.Copy,
                    scale=s4[:, j:j + 1],
                )

                eng = nc.gpsimd if (SPLIT_SCAN_ENGINES and k % 2) else nc.vector
                eng.tensor_tensor_scan(
                    xt[:],
                    a4[:, j:j + 1].to_broadcast((P, t)),
                    xt[:],
                    0.0,
                    OP.mult,
                    OP.add,
                )
                nc.sync.dma_start(yr[k * P:(k + 1) * P, :], xt[:])
    nc.compile()
    return nc


_NC_CACHE = []


def kernel(x, weights, _run_kwargs=None):
    if not _NC_CACHE:
        _NC_CACHE.append(build())
    nc = _NC_CACHE[0]
    x = np.ascontiguousarray(np.asarray(x, dtype=np.float32))
    weights = np.ascontiguousarray(np.asarray(weights, dtype=np.float32))
    in_maps = [
        {"x": x[i * B_PER:(i + 1) * B_PER], "weights": weights}
        for i in range(N_CORES)
    ]
    res = run_bass_kernel_spmd(
        nc, in_maps, core_ids=list(range(N_CORES)), **(_run_kwargs or {})
    )
    out = np.concatenate([res.results[i]["out"] for i in range(N_CORES)], axis=0)
    if _run_kwargs:
        kernel.last_results = res
    return out


# revision 8
# speedup vs baseline: 1.3290x; 1.3235x over previous
"""EMA (first-order linear recurrence along T) for x[16, 512, 4096] f32.

y[..., 0] = x[..., 0];  y[..., t] = s_c * x[..., t] + (1 - s_c) * y[..., t-1]

Sharding: data-parallel over batch B across 8 cores (2 batches/core, each a
contiguous 16 MiB slab). Per core the (b, c) pairs form 1024 independent rows
of length T=4096; the recurrence maps 1:1 onto the TensorTensorScanArith
instruction (state = data0*state + data1 along the free dim, one recurrence
per partition).

Pipeline per 128-row block, all in-place on one SBUF tile X:
  DMA in -> ACT: X[:,1:] *= s (per-partition scale, scalar engine)
         -> scan: X = a*state + X with initial=0 (col 0 still holds raw x_0,
            so state_0 = x_0 exactly) -> DMA out.
Scans alternate Vector/GpSimd so neither engine reaches the DMA roofline
(~94 us for 33.5 MB per core); a lone Vector doing all 8 scans would sit at
~93 us busy and fight the DMA for the critical path.
"""

import numpy as np

import concourse.bacc as bacc
import concourse.mybir as mybir
import concourse.tile as tile
from concourse.bass_utils import run_bass_kernel_spmd

B, C, T = 16, 512, 4096
N_CORES = 8
B_PER = B // N_CORES          # 2 batches per core
ROWS = B_PER * C              # 1024 (b, c) rows per core
P = 128                       # SBUF partitions
N_BLOCKS = ROWS // P          # 8 row blocks per core
C_BLOCKS = C // P             # 4 channel blocks (weights layout)

DT = mybir.dt.float32
OP = mybir.AluOpType

SPLIT_SCAN_ENGINES = False    # GpSimd lacks the scan opcode on trn2 (ISA check)
BUFS = 8


def build(b_per=B_PER, c=C, t=T):
    rows = b_per * c
    n_blocks = rows // P
    c_blocks = c // P

    nc = bacc.Bacc("TRN2", target_bir_lowering=False, debug=False)

    x_in = nc.dram_tensor("x", [b_per, c, t], DT, kind="ExternalInput")
    w_in = nc.dram_tensor("weights", [c], DT, kind="ExternalInput")
    y_out = nc.dram_tensor("out", [b_per, c, t], DT, kind="ExternalOutput")

    xr = x_in.ap().rearrange("b c t -> (b c) t")   # [rows, t]
    yr = y_out.ap().rearrange("b c t -> (b c) t")
    # w4[p, j] = weights[j*128 + p] — column j holds channel block j
    wr = w_in.ap().rearrange("(j p) -> p j", p=P)  # [128, c_blocks]

    with tile.TileContext(nc) as tc:
        with (
            tc.tile_pool(name="const", bufs=1) as cpool,
            tc.tile_pool(name="xp", bufs=BUFS) as xpool,
        ):
            w4 = cpool.tile([P, c_blocks], DT)
            s4 = cpool.tile([P, c_blocks], DT)
            a4 = cpool.tile([P, c_blocks], DT)
            nc.sync.dma_start(w4[:], wr)
            # s = clamp(w, 0, 1); a = 1 - s  (gpsimd: keeps Vector scan-only)
            nc.gpsimd.tensor_scalar(s4[:], w4[:], 0.0, 1.0, OP.max, OP.min)
            nc.gpsimd.tensor_scalar(a4[:], s4[:], -1.0, 1.0, OP.mult, OP.add)

            for k in range(n_blocks):
                j = k % c_blocks  # channel block of rows [k*128, (k+1)*128)
                xt = xpool.tile([P, t], DT)
                nc.sync.dma_start(xt[:], xr[k * P:(k + 1) * P, :])

                # Premultiply s*x in place, skipping col 0: the scan's t=0
                # step then computes state_0 = a*0 + x_0 = x_0 exactly.
                nc.scalar.activation(
                    xt[:, 1:t], xt[:, 1:t], mybir.ActivationFunctionType.Copy,
                    scale=s4[:, j:j + 1],
                )

                eng = nc.gpsimd if (SPLIT_SCAN_ENGINES and k % 2) else nc.vector
                eng.tensor_tensor_scan(
                    xt[:],
                    a4[:, j:j + 1].to_broadcast((P, t)),
                    xt[:],
                    0.0,
                    OP.mult,
                    OP.add,
                )
                # Out-DMAs issue from gpsimd: its own issue queue, so an
                # out (blocked on scan k) never head-of-line-blocks the
                # remaining in-DMAs on sync.
                nc.gpsimd.dma_start(yr[k * P:(k + 1) * P, :], xt[:])
    nc.compile()
    return nc


_NC_CACHE = []


def kernel(x, weights, _run_kwargs=None):
    if not _NC_CACHE:
        _NC_CACHE.append(build())
    nc = _NC_CACHE[0]
    x = np.ascontiguousarray(np.asarray(x, dtype=np.float32))
    weights = np.ascontiguousarray(np.asarray(weights, dtype=np.float32))
    in_maps = [
        {"x": x[i * B_PER:(i + 1) * B_PER], "weights": weights}
        for i in range(N_CORES)
    ]
    res = run_bass_kernel_spmd(
        nc, in_maps, core_ids=list(range(N_CORES)), **(_run_kwargs or {})
    )
    out = np.concatenate([res.results[i]["out"] for i in range(N_CORES)], axis=0)
    if _run_kwargs:
        kernel.last_results = res
    return out
